# revision 3
# baseline (speedup 1.0000x reference)
"""CASSI adjoint (gather shifted bands + mask) as a Bass/Tile SPMD kernel
on 8 Trainium2 NeuronCores.

Reference computation (shapes hardcoded for H=W=1024, L=28, PAD=32):
    out[0, l, h, w] = y_1hw[0, dy[l] + h, dx[l] + w] * mask2d[h, w]
with integer offsets dx/dy derived from phi_d_deg and s_nom on the host.

Sharding: the H (row) dimension is split across the 8 cores — every core
runs an identical program (all 28 bands, offsets baked in as compile-time
constants) over its own 128-row chunk of y/mask/out. Zero communication.
"""

import numpy as np

import concourse.bass as bass  # noqa: F401  (registers AP machinery)
import concourse.mybir as mybir
from concourse import bacc, tile
from concourse.bass_utils import run_bass_kernel_spmd

PI = 3.141592653589793

H, W, L = 1024, 1024, 28
HP, WP = 1056, 1056  # padded input extents (H+PAD, W+PAD)
NCORES = 8
RC = H // NCORES  # 128 rows per core

_cache: dict = {}


def _offsets(phi_d_deg, s_nom):
    """Integer dispersion offsets, mirroring the f32 arithmetic of the
    reference (round-half-to-even, then dynamic_slice start clamping)."""
    phi = np.float32(np.asarray(phi_d_deg, dtype=np.float32).reshape(-1)[0])
    phi_rad = np.float32(phi * np.float32(PI / 180.0))
    s = np.asarray(s_nom, dtype=np.float32)
    dx_f = (s * np.float32(np.cos(phi_rad))).astype(np.float32)
    dy_f = (s * np.float32(np.sin(phi_rad))).astype(np.float32)
    dx_f = (dx_f - dx_f.min()).astype(np.float32)
    dy_f = (dy_f - dy_f.min()).astype(np.float32)
    dx = np.round(dx_f).astype(np.int32)
    dy = np.round(dy_f).astype(np.int32)
    dx = np.clip(dx, 0, WP - W)
    dy = np.clip(dy, 0, HP - H)
    return dx, dy


def _group_schedule(n):
    """Band group sizes: small groups first (fill the store pipeline fast)
    and last (shrink the exposed final store), large in the middle."""
    sizes = [1, 1, 2]
    rem = n - 4  # minus head groups and the final [1, 1] tail
    mid = []
    while rem > 2:
        take = min(4, rem - 2)
        mid.append(take)
        rem -= take
    return sizes + mid + [1, 1]


def _build(dx, dy, obufs=4):
    """Build + compile the per-core program for the given band offsets."""
    max_dy = int(dy.max())
    nrows = RC + max_dy
    nc = bacc.Bacc("TRN2", target_bir_lowering=False, debug=False,
                   num_devices=NCORES)
    f32 = mybir.dt.float32
    y_in = nc.dram_tensor("y_loc", [nrows, WP], f32, kind="ExternalInput")
    m_in = nc.dram_tensor("mask_loc", [RC, W], f32, kind="ExternalInput")
    o_out = nc.dram_tensor("out_loc", [L, RC, W], f32, kind="ExternalOutput")

    sizes = _group_schedule(L)
    assert sum(sizes) == L
    max_g = max(sizes)

    with tile.TileContext(nc) as tc:
        with (
            tc.tile_pool(name="singles", bufs=1) as singles,
            tc.tile_pool(name="ob", bufs=obufs) as obp,
        ):
            # y on the SP HWDGE ring, mask concurrently on the ACT ring.
            ytiles = {}
            for d in sorted({int(v) for v in dy}):
                yt = singles.tile([RC, WP], f32, tag=f"y{d}", name=f"y{d}")
                nc.sync.dma_start(out=yt[:, :], in_=y_in[d : d + RC, :])
                ytiles[d] = yt
            mt = singles.tile([RC, W], f32, tag="mask", name="mask")
            nc.scalar.dma_start(out=mt[:, :], in_=m_in[:, :])

            g0 = 0
            for gi, gsz in enumerate(sizes):
                ot = obp.tile([RC, max_g * W], f32, tag="obuf", name=f"ob{g0}")
                # One fused strided tensor_tensor per run of bands with a
                # uniform dy and a constant dx step; else per-band muls.
                j = 0
                while j < gsz:
                    l = g0 + j
                    d0, x0 = int(dy[l]), int(dx[l])
                    run = 1
                    if j + 1 < gsz:
                        step = int(dx[l + 1]) - x0
                        while (
                            j + run < gsz
                            and int(dy[l + run]) == d0
                            and int(dx[l + run]) - int(dx[l + run - 1]) == step
                        ):
                            run += 1
                    ys = ytiles[d0]
                    if run == 1:
                        nc.vector.tensor_mul(
                            ot[:, j * W : (j + 1) * W],
                            ys[:, x0 : x0 + W],
                            mt[:, :],
                        )
                    else:
                        ysap = ys[:, :]
                        mtap = mt[:, :]
                        in0 = bass.AP(
                            tensor=ysap.tensor,
                            offset=ysap.offset + x0,
                            ap=[ysap.ap[0], [step, run], [1, W]],
                        )
                        in1 = bass.AP(
                            tensor=mtap.tensor,
                            offset=mtap.offset,
                            ap=[mtap.ap[0], [0, run], [1, W]],
                        )
                        outp = ot[:, j * W : (j + run) * W].rearrange(
                            "p (l w) -> p l w", w=W
                        )
                        nc.vector.tensor_mul(outp, in0, in1)
                    j += run
                dview = o_out[g0 : g0 + gsz, :, :].rearrange("l h w -> h l w")
                sview = ot[:, : gsz * W].rearrange("h (l w) -> h l w", w=W)
                eng = nc.sync if gi % 2 == 0 else nc.scalar
                eng.dma_start(out=dview, in_=sview)
                g0 += gsz

    nc.compile()
    return nc


def _run(inputs, trace=False):
    y = np.ascontiguousarray(np.asarray(inputs["y_1hw"], dtype=np.float32)[0])
    mask = np.ascontiguousarray(np.asarray(inputs["mask2d"], dtype=np.float32))
    assert y.shape == (HP, WP) and mask.shape == (H, W)
    dx, dy = _offsets(inputs["phi_d_deg"], inputs["s_nom"])
    assert len(dx) == L

    key = (tuple(dx.tolist()), tuple(dy.tolist()))
    if key not in _cache:
        _cache[key] = _build(dx, dy)
    nc = _cache[key]

    max_dy = int(dy.max())
    in_maps = []
    for c in range(NCORES):
        h0 = c * RC
        in_maps.append(
            {
                "y_loc": np.ascontiguousarray(y[h0 : h0 + RC + max_dy, :]),
                "mask_loc": np.ascontiguousarray(mask[h0 : h0 + RC, :]),
            }
        )
    res = run_bass_kernel_spmd(nc, in_maps, core_ids=list(range(NCORES)),
                               trace=trace)
    out = np.empty((1, L, H, W), dtype=np.float32)
    for c in range(NCORES):
        out[0, :, c * RC : (c + 1) * RC, :] = res.results[c]["out_loc"]
    return out, res


def kernel(**inputs) -> np.ndarray:
    out, _ = _run(inputs)
    return out
